# revision 6
# baseline (speedup 1.0000x reference)
"""MoE layer (E=8 experts, top-2 routing) on 8 Trainium2 NeuronCores.

Strategy: expert-parallel with a 2-slot load-balancing template. The host
computes the gating network in fp64 (logits = x @ wg + bg, top-2, softmax)
and dispatches token-slots to cores. Each core's SPMD program processes
  slot0: NT0 tokens with weight set A (the core's primary expert), then
  slot1: NT1=128 tokens with weight set B (a top-up block of whichever
         expert overflowed NT0 tokens -- host-assigned).
This pads every core to NT0+128 tokens instead of the global max expert
count rounded up (4224 vs 4480 for the reference input), cutting PE time.

Per core FFN:  y = relu(x_e @ w1[e] + b1[e]) @ w2[e], then rows scaled by
the gate weight on-device; the host scatter-adds the two slots per token
back together (plus the combine@b2 bias term).

DMA plan (two HWDGE FIFO rings, serviced round-robin per packet by the 16
SDMA engines):
  scalar ring: x chunk0, w1a fb-block0, b1a, gates, w2a   (critical path)
  sync ring:   w1a fb1-7, x chunks 1.., w1b, w2b, b1b, then output stores
w1a is split into 8 per-fb SBUF tiles so the first matmul only depends on
a 128KB block + the first x chunk (~0.6MB) instead of the full 1.5MB.

All device inputs are host-permuted so every SBUF partition's data is one
contiguous DRAM run. Hardcoded problem shape: x [4,4096,512],
w1 [8,512,1024], w2 [8,1024,512], wg [512,8], top_k=2.
"""

import os
import numpy as np

B, S, D, F, E = 4, 4096, 512, 1024, 8
TOP_K = 2
N_CORES = 8
KD = D // 128   # contraction blocks for mm1
FB = F // 128   # F blocks (h partition blocks / mm2 contraction blocks)
NT1 = 128       # top-up slot tokens

TRACE = os.environ.get("MOE_TRACE", "0") == "1"

_PROGRAM_CACHE = {}


def _chunk_plan(NT0):
    """Token chunk sizes: slot0 split into 512-token chunks (+128-multiple
    remainder), then the 128-token top-up chunk last (small tail)."""
    chunks = [512] * (NT0 // 512)
    if NT0 % 512:
        chunks.append(NT0 % 512)
    chunks.append(NT1)
    return chunks


def _build_program(NT0):
    from concourse import bacc, tile, mybir

    dt = mybir.dt
    DT = dt.bfloat16

    nc = bacc.Bacc("TRN2", target_bir_lowering=False, debug=False)

    chunks = _chunk_plan(NT0)
    offs = [sum(chunks[:i]) for i in range(len(chunks) + 1)]
    NT = NT0 + NT1
    NTG = NT // 128
    n0 = len(chunks) - 1  # number of slot0 chunks

    # host-permuted inputs: per-partition contiguous runs
    # xp: per chunk c, [128, KD*cs] block at col KD*offs[c]
    xp_d = nc.dram_tensor("xp", [128, KD * NT], DT, kind="ExternalInput").ap()
    # w1a/w1b: fb-major: col = fb*(KD*128) + kc*128 + j
    w1a_d = nc.dram_tensor("w1a", [128, FB * KD * 128], DT, kind="ExternalInput").ap()
    w1b_d = nc.dram_tensor("w1b", [128, FB * KD * 128], DT, kind="ExternalInput").ap()
    # w2a/w2b: col = fb*D + d, partition p = f within fb block
    w2a_d = nc.dram_tensor("w2a", [128, FB * D], DT, kind="ExternalInput").ap()
    w2b_d = nc.dram_tensor("w2b", [128, FB * D], DT, kind="ExternalInput").ap()
    b1a_d = nc.dram_tensor("b1a", [128, FB], dt.float32, kind="ExternalInput").ap()
    b1b_d = nc.dram_tensor("b1b", [128, FB], dt.float32, kind="ExternalInput").ap()
    g_d = nc.dram_tensor("gate2", [128, NTG], dt.float32, kind="ExternalInput").ap()
    y_d = nc.dram_tensor("y", [NT, D], DT, kind="ExternalOutput").ap()

    with tile.TileContext(nc) as tc:
        with (
            tc.tile_pool(name="sb", bufs=1) as sbpool,
            tc.tile_pool(name="ps", bufs=4, space="PSUM") as pspool,
        ):
            wpool = xpool = sbpool
            ps1 = ps2 = pspool
            # ---- head DMA plan. Each dma_start costs ~600ns of serial
            # descriptor-generation (DIRECT2D) on its issuing sequencer and
            # the SDMA engines only begin executing at ~8.2us, so the head
            # is gen-count-limited: put the first-needed tiles first, one
            # tile per gen, and group later weight blocks into few gens.
            # Critical first matmul deps: w1afb0 (scalar #1) + x0k0 (sync
            # #1), both 128KB.
            cs0 = chunks[0]
            w1afb0 = wpool.tile([128, KD * 128], DT, tag="w1a_fb0",
                                name="w1afb0")
            nc.scalar.dma_start(out=w1afb0[:], in_=w1a_d[:, 0:KD * 128])
            x0_kc = [None] * KD
            for kc, eng in ((0, nc.sync), (1, nc.scalar), (2, nc.sync),
                            (3, nc.scalar)):
                xt = xpool.tile([128, cs0], DT, tag=f"x0k{kc}", name=f"x0k{kc}")
                eng.dma_start(out=xt[:], in_=xp_d[:, kc * cs0:(kc + 1) * cs0])
                x0_kc[kc] = xt
            # w1a fb1-7 grouped into 3 gens on sync, sized so each lands
            # just ahead of its first consuming chain
            w1afb1 = wpool.tile([128, KD * 128], DT, name="w1afb1")
            nc.sync.dma_start(out=w1afb1[:], in_=w1a_d[:, KD * 128:2 * KD * 128])
            w1afb23 = wpool.tile([128, 2 * KD * 128], DT, name="w1afb23")
            nc.sync.dma_start(out=w1afb23[:],
                              in_=w1a_d[:, 2 * KD * 128:4 * KD * 128])
            w1afb47 = wpool.tile([128, 4 * KD * 128], DT, name="w1afb47")
            nc.sync.dma_start(out=w1afb47[:],
                              in_=w1a_d[:, 4 * KD * 128:8 * KD * 128])

            def w1a_slice(fb, kc):
                if fb == 0:
                    return w1afb0[:, kc * 128:(kc + 1) * 128]
                if fb == 1:
                    return w1afb1[:, kc * 128:(kc + 1) * 128]
                if fb < 4:
                    base = (fb - 2) * KD * 128
                    return w1afb23[:, base + kc * 128:base + (kc + 1) * 128]
                base = (fb - 4) * KD * 128
                return w1afb47[:, base + kc * 128:base + (kc + 1) * 128]

            # scalar ring continues: tiny b1a/g (needed by the first RELU),
            # then w2a for the first mm2
            b1a_sb = wpool.tile([128, FB], dt.float32)
            nc.scalar.dma_start(out=b1a_sb[:], in_=b1a_d[:])
            g_sb = wpool.tile([128, NTG], dt.float32)
            nc.scalar.dma_start(out=g_sb[:], in_=g_d[:])
            w2a_A = wpool.tile([128, (FB // 2) * D], DT, name="w2aA")
            nc.scalar.dma_start(out=w2a_A[:], in_=w2a_d[:, 0:(FB // 2) * D])

            # warmup: dummy matmuls on a zeroed scratch tile keep the PE
            # busy from the engine-init floor (~8.2us) through the DVFS
            # ramp until the first x/w tiles land (~9us); the scratch psum
            # is never read. Overshoot costs one small dummy.
            warm = wpool.tile([128, 512], DT)
            nc.gpsimd.memset(warm[:], 0.0)
            for i in range(6):
                pw = ps2.tile([128, 512], dt.float32, tag="ps2")
                if i < 3:
                    nc.tensor.matmul(pw[:], warm[:, 0:128], warm[:],
                                     start=True, stop=True)
                else:
                    nc.tensor.matmul(pw[:, 0:128], warm[:, 0:128],
                                     warm[:, 0:128], start=True, stop=True)

            # ---- sync (SP) HWDGE ring: bulk loads continue, stores below.
            x_tiles = [tuple(x0_kc)]

            for c in range(1, len(chunks)):
                cs = chunks[c]
                if c == 1:
                    # two kc-half tiles in the same ring/FIFO slot so
                    # chunk1's mm1 can start on the first half (kc blocks
                    # are read in order)
                    xa = xpool.tile([128, 2 * cs], DT, tag="x1a", name="x1a")
                    nc.sync.dma_start(
                        out=xa[:], in_=xp_d[:, KD * offs[c]:KD * offs[c] + 2 * cs])
                    xb = xpool.tile([128, 2 * cs], DT, tag="x1b", name="x1b")
                    nc.sync.dma_start(
                        out=xb[:],
                        in_=xp_d[:, KD * offs[c] + 2 * cs:KD * (offs[c] + cs)])
                    x_tiles.append((xa, xb))
                elif c == 2:
                    # w2a_B rides between x1 and x2: needed by mm2(c0) which
                    # now runs after mm1(c1), so ~21us of slack
                    w2a_B = wpool.tile([128, (FB // 2) * D], DT, name="w2aB")
                    nc.sync.dma_start(out=w2a_B[:], in_=w2a_d[:, (FB // 2) * D:])
                    xt = xpool.tile([128, KD * cs], DT, tag=f"x{c}", name=f"x{c}")
                    nc.sync.dma_start(
                        out=xt[:], in_=xp_d[:, KD * offs[c]:KD * (offs[c] + cs)])
                    x_tiles.append(xt)
                else:
                    xt = xpool.tile([128, KD * cs], DT, tag=f"x{c}", name=f"x{c}")
                    nc.sync.dma_start(
                        out=xt[:], in_=xp_d[:, KD * offs[c]:KD * (offs[c] + cs)])
                    x_tiles.append(xt)

            if len(chunks) <= 2:  # tiny-NT0 fallback: w2a_B not yet emitted
                w2a_B = wpool.tile([128, (FB // 2) * D], DT, name="w2aB")
                nc.sync.dma_start(out=w2a_B[:], in_=w2a_d[:, (FB // 2) * D:])

            w1b_sb = wpool.tile([128, FB * KD * 128], DT)
            nc.sync.dma_start(out=w1b_sb[:], in_=w1b_d[:])
            w2b_sb = wpool.tile([128, FB * D], DT)
            nc.sync.dma_start(out=w2b_sb[:], in_=w2b_d[:])
            b1b_sb = wpool.tile([128, FB], dt.float32)
            nc.sync.dma_start(out=b1b_sb[:], in_=b1b_d[:])

            def w1_slice(c, fb, kc):
                if c < n0:
                    return w1a_slice(fb, kc)
                return w1b_sb[:, fb * KD * 128 + kc * 128:fb * KD * 128 + (kc + 1) * 128]

            def w2_slice(c, fb, c0=0, c1=D):
                if c >= n0:
                    return w2b_sb[:, fb * D + c0:fb * D + c1]
                t, f = (w2a_A, fb) if fb < FB // 2 else (w2a_B, fb - FB // 2)
                return t[:, f * D + c0:f * D + c1]

            h_tiles = {}

            def do_mm1(c):
                cs = chunks[c]
                x_sb = x_tiles[c]
                b1_sb = b1a_sb if c < n0 else b1b_sb
                h_sb = sbpool.tile([128, FB, cs], DT, tag="h", bufs=4)
                h_tiles[c] = h_sb
                for fb in range(FB):
                    p = ps1.tile([128, cs], dt.float32, tag="ps1")
                    for kc in range(KD):
                        if isinstance(x_sb, tuple) and len(x_sb) == KD:
                            xop = x_sb[kc][:]
                        elif isinstance(x_sb, tuple):
                            xt_, k_ = (x_sb[0], kc) if kc < 2 else (x_sb[1], kc - 2)
                            xop = xt_[:, k_ * cs:(k_ + 1) * cs]
                        else:
                            xop = x_sb[:, kc * cs:(kc + 1) * cs]
                        nc.tensor.matmul(
                            p[:],
                            w1_slice(c, fb, kc),
                            xop,
                            start=(kc == 0),
                            stop=(kc == KD - 1),
                        )
                    nc.scalar.activation(
                        h_sb[:, fb, :],
                        p[:],
                        mybir.ActivationFunctionType.Relu,
                        bias=b1_sb[:, fb:fb + 1],
                        scale=1.0,
                    )

            def do_mm2(c):
                cs = chunks[c]
                h_sb = h_tiles.pop(c)
                last_chunk = c == len(chunks) - 1
                for tb in range(cs // 128):
                    blk = offs[c] // 128 + tb
                    r0 = offs[c] + tb * 128
                    if not (last_chunk and tb == cs // 128 - 1):
                        p2 = ps2.tile([128, 512], dt.float32, tag="ps2")
                        for fb in range(FB):
                            nc.tensor.matmul(
                                p2[:],
                                h_sb[:, fb, tb * 128:(tb + 1) * 128],
                                w2_slice(c, fb),
                                start=(fb == 0),
                                stop=(fb == FB - 1),
                            )
                        o_sb = sbpool.tile([128, 512], DT, tag="o", bufs=10)
                        nc.vector.tensor_scalar_mul(
                            o_sb[:], p2[:], g_sb[:, blk:blk + 1]
                        )
                        nc.sync.dma_start(out=y_d[r0:r0 + 128, :], in_=o_sb[:])
                    else:
                        # final 128-token block: column-split mm2 into two
                        # halves so the gate-scale + store of half A overlap
                        # mm2 of half B, shrinking the post-last-matmul tail
                        for half, eng in ((0, nc.sync), (1, nc.scalar)):
                            # reuse the regular ps2 slots ([128,512] tag) so
                            # PSUM stays within the 8-bank budget
                            p2 = ps2.tile([128, 512], dt.float32, tag="ps2")
                            for fb in range(FB):
                                nc.tensor.matmul(
                                    p2[:, 0:256],
                                    h_sb[:, fb, tb * 128:(tb + 1) * 128],
                                    w2_slice(c, fb, half * 256, (half + 1) * 256),
                                    start=(fb == 0),
                                    stop=(fb == FB - 1),
                                )
                            o_sb = sbpool.tile([128, 256], DT, tag="oh", bufs=4)
                            nc.vector.tensor_scalar_mul(
                                o_sb[:], p2[:, 0:256], g_sb[:, blk:blk + 1]
                            )
                            eng.dma_start(
                                out=y_d[r0:r0 + 128, half * 256:(half + 1) * 256],
                                in_=o_sb[:],
                            )

            # software pipeline: mm1 runs one chunk ahead of mm2, so the
            # first mm2's w2a dependency has ~2 chunk-times of DMA slack
            nchunks = len(chunks)
            for c in range(nchunks):
                do_mm1(c)
                if c >= 1:
                    do_mm2(c - 1)
            do_mm2(nchunks - 1)
    nc.compile()
    return nc


def _install_ntff_hook():
    """Register the axon NTFF profiling hook that run_bass_kernel_spmd
    (trace=True) looks for under antenv.axon_hooks; this container's antenv
    lacks that module, so recreate it via ctypes against libaxon_pjrt.so."""
    import sys, types, ctypes, contextlib

    if "antenv.axon_hooks" in sys.modules:
        return
    try:
        lib = ctypes.CDLL("/opt/axon/libaxon_pjrt.so")
    except OSError:
        return
    if not hasattr(lib, "axon_start_nrt_profile"):
        return
    lib.axon_start_nrt_profile.argtypes = [ctypes.POINTER(ctypes.c_int64), ctypes.c_size_t]
    lib.axon_start_nrt_profile.restype = ctypes.c_int64
    lib.axon_stop_nrt_profile.argtypes = [ctypes.c_char_p]
    lib.axon_stop_nrt_profile.restype = ctypes.c_int64

    @contextlib.contextmanager
    def _hook(output_dir, device_ids):
        import jax

        jax.devices()
        if device_ids:
            ids = (ctypes.c_int64 * len(device_ids))(*device_ids)
            rc = lib.axon_start_nrt_profile(ids, len(device_ids))
        else:
            rc = lib.axon_start_nrt_profile(None, 0)
        if rc != 0:
            raise RuntimeError(f"axon_start_nrt_profile rc={rc}")
        try:
            yield
        finally:
            n = lib.axon_stop_nrt_profile(str(output_dir).encode())
            print(f"profile: {n} ntff file(s) written to {output_dir}")

    mod = types.ModuleType("antenv.axon_hooks")
    _holder = {"h": _hook}
    mod.set_axon_ntff_profile_hook = lambda h: _holder.__setitem__("h", h)
    mod.get_axon_ntff_profile_hook = lambda: _holder["h"]
    sys.modules["antenv.axon_hooks"] = mod

    # avoid the S3/Fish artifact upload in the trace post-processing path
    import concourse.bass_utils as bu

    bu.upload_artifacts = lambda tmpdir: str(tmpdir)


def _pick_nt0(counts):
    """Smallest NT0 (multiple of 128) such that the overflow of every
    expert beyond NT0 fits in the 8 per-core 128-token top-up slots.
    Compare against the no-top-up template (pad all to max count)."""
    cmax = int(counts.max())
    nt_plain = max(512, -(-cmax // 128) * 128)
    best = None
    for nt0 in range(512, nt_plain + 128, 128):
        need = sum(-(-max(0, int(c) - nt0) // NT1) for c in counts)
        if need <= N_CORES:
            best = nt0
            break
    if best is None or best + NT1 >= nt_plain + NT1:
        best = nt_plain  # top-ups unused (gate=0 padding)
    return best


def kernel(**inputs):
    from concourse.bass_utils import run_bass_kernel_spmd
    import ml_dtypes

    if TRACE:
        _install_ntff_hook()

    x = np.asarray(inputs["x"], np.float32)
    w1 = np.asarray(inputs["w1"], np.float32)
    b1 = np.asarray(inputs["b1"], np.float32)
    w2 = np.asarray(inputs["w2"], np.float32)
    b2 = np.asarray(inputs["b2"], np.float32)
    wg = np.asarray(inputs["wg"], np.float32)
    bg = np.asarray(inputs["bg"], np.float32)

    T = x.shape[0] * x.shape[1]
    xf = x.reshape(T, D)

    # ---- host gating (fp64): logits -> top-2 (jax.lax.top_k tie order:
    # lower index wins -> stable argsort on -logits) -> softmax over top-2.
    logits = xf.astype(np.float64) @ wg.astype(np.float64) + bg.astype(np.float64)
    order = np.argsort(-logits, axis=1, kind="stable")
    top_idx = order[:, :TOP_K]                      # [T, K]
    top_vals = np.take_along_axis(logits, top_idx, axis=1)
    gwts = np.exp(top_vals - top_vals.max(axis=1, keepdims=True))
    gwts = gwts / gwts.sum(axis=1, keepdims=True)   # [T, K]

    # ---- dispatch: sort slots (t, k) by expert; per-expert contiguous runs.
    flat_expert = top_idx.ravel()                   # slot s = t*K + k
    perm = np.argsort(flat_expert, kind="stable")   # slots grouped by expert
    counts = np.bincount(flat_expert, minlength=E)
    cum = np.concatenate([[0], np.cumsum(counts)])
    slot_tok = perm // TOP_K                        # token of each sorted slot
    gates_sorted = gwts.ravel()[perm].astype(np.float32)

    NT0 = _pick_nt0(counts)
    NT = NT0 + NT1
    NTG = NT // 128
    chunks = _chunk_plan(NT0)
    offs = [sum(chunks[:i]) for i in range(len(chunks) + 1)]

    io_dtype = ml_dtypes.bfloat16
    w1_io = w1.astype(io_dtype)
    w2_io = w2.astype(io_dtype)

    # top-up assignment: expert e's slots beyond NT0, chopped into
    # 128-blocks, each block -> one core's slot1. record: (core, e, lo, n)
    topups = []
    next_core = 0
    for e in range(E):
        n = int(counts[e])
        for lo in range(NT0, n, NT1):
            nb = min(NT1, n - lo)
            assert next_core < N_CORES, "top-up slots exhausted"
            topups.append((next_core, e, lo, nb))
            next_core += 1
    topup_by_core = {c: (e, lo, nb) for (c, e, lo, nb) in topups}

    def permute_x(xt):
        # xt [D, NT] -> [128, KD*NT]: per chunk, (kc, token) contiguous
        xr = xt.reshape(KD, 128, NT)
        parts = [
            xr[:, :, offs[c]:offs[c + 1]].transpose(1, 0, 2).reshape(128, -1)
            for c in range(len(chunks))
        ]
        return np.ascontiguousarray(np.concatenate(parts, axis=1))

    def pack_w1(e):
        # [128, FB*KD*128] fb-major: col = fb*KD*128 + kc*128 + j
        w = w1_io[e].reshape(KD, 128, FB, 128)       # [kc, p, fb, j]
        return np.ascontiguousarray(
            w.transpose(1, 2, 0, 3).reshape(128, FB * KD * 128))

    def pack_w2(e):
        return np.ascontiguousarray(
            w2_io[e].reshape(FB, 128, D).transpose(1, 0, 2).reshape(128, FB * D))

    def pack_b1(e):
        return np.ascontiguousarray(b1[e].reshape(FB, 128).T)

    in_maps = []
    for c in range(N_CORES):
        n0 = min(int(counts[c]), NT0)
        toks0 = slot_tok[cum[c]:cum[c] + n0]
        xt = np.zeros((D, NT), io_dtype)
        xt[:, :n0] = xf[toks0].astype(io_dtype).T
        gate = np.zeros(NT, np.float32)
        gate[:n0] = gates_sorted[cum[c]:cum[c] + n0]
        if c in topup_by_core:
            te, lo, nb = topup_by_core[c]
            tt = slot_tok[cum[te] + lo:cum[te] + lo + nb]
            xt[:, NT0:NT0 + nb] = xf[tt].astype(io_dtype).T
            gate[NT0:NT0 + nb] = gates_sorted[cum[te] + lo:cum[te] + lo + nb]
            eb = te
        else:
            eb = 0  # unused slot1: gate=0 rows, any weights
        in_maps.append({
            "xp": permute_x(xt),
            "w1a": pack_w1(c), "w2a": pack_w2(c), "b1a": pack_b1(c),
            "w1b": pack_w1(eb), "w2b": pack_w2(eb), "b1b": pack_b1(eb),
            "gate2": np.ascontiguousarray(gate.reshape(NTG, 128).T),
        })

    def run_device():
        if NT0 not in _PROGRAM_CACHE:
            _PROGRAM_CACHE[NT0] = _build_program(NT0)
        nc = _PROGRAM_CACHE[NT0]
        res = run_bass_kernel_spmd(nc, in_maps, list(range(N_CORES)), trace=TRACE)
        if TRACE and res.exec_time_ns is not None:
            print(f"HW exec time: {res.exec_time_ns} ns")
        return [res.results[c]["y"] for c in range(N_CORES)]

    try:
        try:
            y_cores = run_device()
        except Exception:
            # transient device errors (e.g. NRT exec-unit unrecoverable)
            # are usually gone on retry with a freshly built program
            _PROGRAM_CACHE.clear()
            y_cores = run_device()
    except Exception as exc:
        # last resort: identical math on the host so the result is still
        # correct even if the accelerator path is down
        import sys
        print(f"device path failed twice ({exc!r}); computing FFN on host",
              file=sys.stderr)
        out_slots = np.zeros((T * TOP_K, D), np.float32)
        for e in range(E):
            n = int(counts[e])
            toks = slot_tok[cum[e]:cum[e] + n]
            h = np.maximum(xf[toks] @ w1[e] + b1[e], 0.0)
            y = (h @ w2[e]) * gates_sorted[cum[e]:cum[e] + n, None]
            out_slots[perm[cum[e]:cum[e] + n]] = y.astype(np.float32)
        out = out_slots.reshape(T, TOP_K, D).sum(axis=1)
        combine = np.zeros((T, E), np.float32)
        np.put_along_axis(combine, top_idx, gwts.astype(np.float32), axis=1)
        out += combine @ b2
        return out.reshape(B, S, D).astype(np.float32)

    # ---- unshard: scatter slots back, sum the K slots per token, add b2.
    out_slots = np.zeros((T * TOP_K, D), np.float32)
    for c in range(N_CORES):
        n0 = min(int(counts[c]), NT0)
        out_slots[perm[cum[c]:cum[c] + n0]] = \
            y_cores[c][:n0].astype(np.float32)
    for (c, e, lo, nb) in topups:
        out_slots[perm[cum[e] + lo:cum[e] + lo + nb]] = \
            y_cores[c][NT0:NT0 + nb].astype(np.float32)
    out = out_slots.reshape(T, TOP_K, D).sum(axis=1)

    # combine @ b2 (gate-weighted expert output biases)
    combine = np.zeros((T, E), np.float32)
    np.put_along_axis(combine, top_idx, gwts.astype(np.float32), axis=1)
    out += combine @ b2

    return out.reshape(B, S, D).astype(np.float32)



# revision 7
# speedup vs baseline: 1.1918x; 1.1918x over previous
"""MoE layer (E=8 experts, top-2 routing) on 8 Trainium2 NeuronCores.

Strategy: expert-parallel with a 2-slot load-balancing template. The host
computes the gating network in fp64 (logits = x @ wg + bg, top-2, softmax)
and dispatches token-slots to cores. Each core's SPMD program processes
  slot0: NT0 tokens with weight set A (the core's primary expert), then
  slot1: NT1=128 tokens with weight set B (a top-up block of whichever
         expert overflowed NT0 tokens -- host-assigned).
This pads every core to NT0+128 tokens instead of the global max expert
count rounded up (4224 vs 4480 for the reference input), cutting PE time.

Per core FFN:  y = relu(x_e @ w1[e] + b1[e]) @ w2[e], then rows scaled by
the gate weight on-device; the host scatter-adds the two slots per token
back together (plus the combine@b2 bias term).

DMA plan (two HWDGE FIFO rings, serviced round-robin per packet by the 16
SDMA engines):
  scalar ring: x chunk0, w1a fb-block0, b1a, gates, w2a   (critical path)
  sync ring:   w1a fb1-7, x chunks 1.., w1b, w2b, b1b, then output stores
w1a is split into 8 per-fb SBUF tiles so the first matmul only depends on
a 128KB block + the first x chunk (~0.6MB) instead of the full 1.5MB.

All device inputs are host-permuted so every SBUF partition's data is one
contiguous DRAM run. Hardcoded problem shape: x [4,4096,512],
w1 [8,512,1024], w2 [8,1024,512], wg [512,8], top_k=2.
"""

import os
import numpy as np

B, S, D, F, E = 4, 4096, 512, 1024, 8
TOP_K = 2
N_CORES = 8
KD = D // 128   # contraction blocks for mm1
FB = F // 128   # F blocks (h partition blocks / mm2 contraction blocks)
NT1 = 128       # top-up slot tokens

TRACE = os.environ.get("MOE_TRACE", "0") == "1"

_PROGRAM_CACHE = {}


def _chunk_plan(NT0):
    """Token chunk sizes: slot0 split into 512-token chunks (+128-multiple
    remainder), then the 128-token top-up chunk last (small tail)."""
    chunks = [512] * (NT0 // 512)
    if NT0 % 512:
        chunks.append(NT0 % 512)
    chunks.append(NT1)
    return chunks


def _build_program(NT0):
    from concourse import bacc, tile, mybir

    dt = mybir.dt
    DT = dt.bfloat16

    nc = bacc.Bacc("TRN2", target_bir_lowering=False, debug=False)

    chunks = _chunk_plan(NT0)
    offs = [sum(chunks[:i]) for i in range(len(chunks) + 1)]
    NT = NT0 + NT1
    NTG = NT // 128
    n0 = len(chunks) - 1  # number of slot0 chunks

    # host-permuted inputs: per-partition contiguous runs
    # xp: per chunk c, [128, KD*cs] block at col KD*offs[c]
    xp_d = nc.dram_tensor("xp", [128, KD * NT], DT, kind="ExternalInput").ap()
    # w1a/w1b: fb-major: col = fb*(KD*128) + kc*128 + j
    w1a_d = nc.dram_tensor("w1a", [128, FB * KD * 128], DT, kind="ExternalInput").ap()
    w1b_d = nc.dram_tensor("w1b", [128, FB * KD * 128], DT, kind="ExternalInput").ap()
    # w2a/w2b: col = fb*D + d, partition p = f within fb block
    w2a_d = nc.dram_tensor("w2a", [128, FB * D], DT, kind="ExternalInput").ap()
    w2b_d = nc.dram_tensor("w2b", [128, FB * D], DT, kind="ExternalInput").ap()
    b1a_d = nc.dram_tensor("b1a", [128, FB], dt.float32, kind="ExternalInput").ap()
    b1b_d = nc.dram_tensor("b1b", [128, FB], dt.float32, kind="ExternalInput").ap()
    g_d = nc.dram_tensor("gate2", [128, NTG], dt.float32, kind="ExternalInput").ap()
    y_d = nc.dram_tensor("y", [NT, D], DT, kind="ExternalOutput").ap()

    with tile.TileContext(nc) as tc:
        with (
            tc.tile_pool(name="sb", bufs=1) as sbpool,
            tc.tile_pool(name="ps", bufs=4, space="PSUM") as pspool,
        ):
            wpool = xpool = sbpool
            ps1 = ps2 = pspool
            # ---- head DMA plan. Each dma_start costs ~600ns of serial
            # descriptor-generation (DIRECT2D) on its issuing sequencer and
            # the SDMA engines only begin executing at ~8.2us, so the head
            # is gen-count-limited: put the first-needed tiles first, one
            # tile per gen, and group later weight blocks into few gens.
            # Critical first matmul deps: w1afb0 (scalar #1) + x0k0 (sync
            # #1), both 128KB.
            cs0 = chunks[0]
            w1afb0 = wpool.tile([128, KD * 128], DT, tag="w1a_fb0",
                                name="w1afb0")
            nc.scalar.dma_start(out=w1afb0[:], in_=w1a_d[:, 0:KD * 128])
            x0_kc = [None] * KD
            for kc, eng in ((0, nc.sync), (1, nc.scalar), (2, nc.sync),
                            (3, nc.scalar)):
                xt = xpool.tile([128, cs0], DT, tag=f"x0k{kc}", name=f"x0k{kc}")
                eng.dma_start(out=xt[:], in_=xp_d[:, kc * cs0:(kc + 1) * cs0])
                x0_kc[kc] = xt
            # w1a fb1-7 grouped into 3 gens on sync, sized so each lands
            # just ahead of its first consuming chain
            w1afb1 = wpool.tile([128, KD * 128], DT, name="w1afb1")
            nc.sync.dma_start(out=w1afb1[:], in_=w1a_d[:, KD * 128:2 * KD * 128])
            w1afb23 = wpool.tile([128, 2 * KD * 128], DT, name="w1afb23")
            nc.sync.dma_start(out=w1afb23[:],
                              in_=w1a_d[:, 2 * KD * 128:4 * KD * 128])
            w1afb47 = wpool.tile([128, 4 * KD * 128], DT, name="w1afb47")
            nc.sync.dma_start(out=w1afb47[:],
                              in_=w1a_d[:, 4 * KD * 128:8 * KD * 128])

            def w1a_slice(fb, kc):
                if fb == 0:
                    return w1afb0[:, kc * 128:(kc + 1) * 128]
                if fb == 1:
                    return w1afb1[:, kc * 128:(kc + 1) * 128]
                if fb < 4:
                    base = (fb - 2) * KD * 128
                    return w1afb23[:, base + kc * 128:base + (kc + 1) * 128]
                base = (fb - 4) * KD * 128
                return w1afb47[:, base + kc * 128:base + (kc + 1) * 128]

            # scalar ring continues: tiny b1a/g (needed by the first RELU),
            # then w2a for the first mm2
            b1a_sb = wpool.tile([128, FB], dt.float32)
            nc.scalar.dma_start(out=b1a_sb[:], in_=b1a_d[:])
            g_sb = wpool.tile([128, NTG], dt.float32)
            nc.scalar.dma_start(out=g_sb[:], in_=g_d[:])
            w2a_A = wpool.tile([128, (FB // 2) * D], DT, name="w2aA")
            nc.scalar.dma_start(out=w2a_A[:], in_=w2a_d[:, 0:(FB // 2) * D])

            # warmup: dummy matmuls reading the first real weight tile keep
            # the PE busy through the DVFS ramp until chunk0's x blocks
            # land; values are irrelevant, the scratch psum is never read.
            # (No gpsimd memset: engaging gpsimd was measured to depress
            # the PE clock for the whole kernel.)
            for i in range(10):
                pw = ps2.tile([128, 512], dt.float32, tag="ps2")
                if i < 2:
                    nc.tensor.matmul(pw[:], w1afb0[:, 0:128],
                                     w1afb0[:, 0:512], start=True, stop=True)
                else:
                    nc.tensor.matmul(pw[:, 0:128], w1afb0[:, 0:128],
                                     w1afb0[:, 0:128], start=True, stop=True)

            # ---- sync (SP) HWDGE ring: bulk loads continue, stores below.
            x_tiles = [tuple(x0_kc)]

            for c in range(1, len(chunks)):
                cs = chunks[c]
                if c == 1:
                    # two kc-half tiles in the same ring/FIFO slot so
                    # chunk1's mm1 can start on the first half (kc blocks
                    # are read in order)
                    xa = xpool.tile([128, 2 * cs], DT, tag="x1a", name="x1a")
                    nc.sync.dma_start(
                        out=xa[:], in_=xp_d[:, KD * offs[c]:KD * offs[c] + 2 * cs])
                    xb = xpool.tile([128, 2 * cs], DT, tag="x1b", name="x1b")
                    nc.sync.dma_start(
                        out=xb[:],
                        in_=xp_d[:, KD * offs[c] + 2 * cs:KD * (offs[c] + cs)])
                    x_tiles.append((xa, xb))
                elif c == 2:
                    # w2a_B rides between x1 and x2: needed by mm2(c0) which
                    # now runs after mm1(c1), so ~21us of slack
                    w2a_B = wpool.tile([128, (FB // 2) * D], DT, name="w2aB")
                    nc.sync.dma_start(out=w2a_B[:], in_=w2a_d[:, (FB // 2) * D:])
                    xt = xpool.tile([128, KD * cs], DT, tag=f"x{c}", name=f"x{c}")
                    nc.sync.dma_start(
                        out=xt[:], in_=xp_d[:, KD * offs[c]:KD * (offs[c] + cs)])
                    x_tiles.append(xt)
                else:
                    xt = xpool.tile([128, KD * cs], DT, tag=f"x{c}", name=f"x{c}")
                    nc.sync.dma_start(
                        out=xt[:], in_=xp_d[:, KD * offs[c]:KD * (offs[c] + cs)])
                    x_tiles.append(xt)

            if len(chunks) <= 2:  # tiny-NT0 fallback: w2a_B not yet emitted
                w2a_B = wpool.tile([128, (FB // 2) * D], DT, name="w2aB")
                nc.sync.dma_start(out=w2a_B[:], in_=w2a_d[:, (FB // 2) * D:])

            w1b_sb = wpool.tile([128, FB * KD * 128], DT)
            nc.sync.dma_start(out=w1b_sb[:], in_=w1b_d[:])
            w2b_sb = wpool.tile([128, FB * D], DT)
            nc.sync.dma_start(out=w2b_sb[:], in_=w2b_d[:])
            b1b_sb = wpool.tile([128, FB], dt.float32)
            nc.sync.dma_start(out=b1b_sb[:], in_=b1b_d[:])

            def w1_slice(c, fb, kc):
                if c < n0:
                    return w1a_slice(fb, kc)
                return w1b_sb[:, fb * KD * 128 + kc * 128:fb * KD * 128 + (kc + 1) * 128]

            def w2_slice(c, fb, c0=0, c1=D):
                if c >= n0:
                    return w2b_sb[:, fb * D + c0:fb * D + c1]
                t, f = (w2a_A, fb) if fb < FB // 2 else (w2a_B, fb - FB // 2)
                return t[:, f * D + c0:f * D + c1]

            h_tiles = {}

            def do_mm1(c):
                cs = chunks[c]
                x_sb = x_tiles[c]
                b1_sb = b1a_sb if c < n0 else b1b_sb
                h_sb = sbpool.tile([128, FB, cs], DT, tag="h", bufs=4)
                h_tiles[c] = h_sb
                for fb in range(FB):
                    p = ps1.tile([128, cs], dt.float32, tag="ps1")
                    for kc in range(KD):
                        if isinstance(x_sb, tuple) and len(x_sb) == KD:
                            xop = x_sb[kc][:]
                        elif isinstance(x_sb, tuple):
                            xt_, k_ = (x_sb[0], kc) if kc < 2 else (x_sb[1], kc - 2)
                            xop = xt_[:, k_ * cs:(k_ + 1) * cs]
                        else:
                            xop = x_sb[:, kc * cs:(kc + 1) * cs]
                        nc.tensor.matmul(
                            p[:],
                            w1_slice(c, fb, kc),
                            xop,
                            start=(kc == 0),
                            stop=(kc == KD - 1),
                        )
                    nc.scalar.activation(
                        h_sb[:, fb, :],
                        p[:],
                        mybir.ActivationFunctionType.Relu,
                        bias=b1_sb[:, fb:fb + 1],
                        scale=1.0,
                    )

            def do_mm2(c):
                cs = chunks[c]
                h_sb = h_tiles.pop(c)
                last_chunk = c == len(chunks) - 1
                for tb in range(cs // 128):
                    blk = offs[c] // 128 + tb
                    r0 = offs[c] + tb * 128
                    if not (last_chunk and tb == cs // 128 - 1):
                        p2 = ps2.tile([128, 512], dt.float32, tag="ps2")
                        for fb in range(FB):
                            nc.tensor.matmul(
                                p2[:],
                                h_sb[:, fb, tb * 128:(tb + 1) * 128],
                                w2_slice(c, fb),
                                start=(fb == 0),
                                stop=(fb == FB - 1),
                            )
                        o_sb = sbpool.tile([128, 512], DT, tag="o", bufs=10)
                        nc.vector.tensor_scalar_mul(
                            o_sb[:], p2[:], g_sb[:, blk:blk + 1]
                        )
                        nc.sync.dma_start(out=y_d[r0:r0 + 128, :], in_=o_sb[:])
                    else:
                        # final 128-token block: column-split mm2 into two
                        # halves so the gate-scale + store of half A overlap
                        # mm2 of half B, shrinking the post-last-matmul tail
                        for half, eng in ((0, nc.sync), (1, nc.scalar)):
                            # reuse the regular ps2 slots ([128,512] tag) so
                            # PSUM stays within the 8-bank budget
                            p2 = ps2.tile([128, 512], dt.float32, tag="ps2")
                            for fb in range(FB):
                                nc.tensor.matmul(
                                    p2[:, 0:256],
                                    h_sb[:, fb, tb * 128:(tb + 1) * 128],
                                    w2_slice(c, fb, half * 256, (half + 1) * 256),
                                    start=(fb == 0),
                                    stop=(fb == FB - 1),
                                )
                            o_sb = sbpool.tile([128, 256], DT, tag="oh", bufs=4)
                            nc.vector.tensor_scalar_mul(
                                o_sb[:], p2[:, 0:256], g_sb[:, blk:blk + 1]
                            )
                            eng.dma_start(
                                out=y_d[r0:r0 + 128, half * 256:(half + 1) * 256],
                                in_=o_sb[:],
                            )

            # software pipeline: mm1 runs one chunk ahead of mm2, so the
            # first mm2's w2a dependency has ~2 chunk-times of DMA slack
            nchunks = len(chunks)
            for c in range(nchunks):
                do_mm1(c)
                if c >= 1:
                    do_mm2(c - 1)
            do_mm2(nchunks - 1)
    nc.compile()
    return nc


def _install_ntff_hook():
    """Register the axon NTFF profiling hook that run_bass_kernel_spmd
    (trace=True) looks for under antenv.axon_hooks; this container's antenv
    lacks that module, so recreate it via ctypes against libaxon_pjrt.so."""
    import sys, types, ctypes, contextlib

    if "antenv.axon_hooks" in sys.modules:
        return
    try:
        lib = ctypes.CDLL("/opt/axon/libaxon_pjrt.so")
    except OSError:
        return
    if not hasattr(lib, "axon_start_nrt_profile"):
        return
    lib.axon_start_nrt_profile.argtypes = [ctypes.POINTER(ctypes.c_int64), ctypes.c_size_t]
    lib.axon_start_nrt_profile.restype = ctypes.c_int64
    lib.axon_stop_nrt_profile.argtypes = [ctypes.c_char_p]
    lib.axon_stop_nrt_profile.restype = ctypes.c_int64

    @contextlib.contextmanager
    def _hook(output_dir, device_ids):
        import jax

        jax.devices()
        if device_ids:
            ids = (ctypes.c_int64 * len(device_ids))(*device_ids)
            rc = lib.axon_start_nrt_profile(ids, len(device_ids))
        else:
            rc = lib.axon_start_nrt_profile(None, 0)
        if rc != 0:
            raise RuntimeError(f"axon_start_nrt_profile rc={rc}")
        try:
            yield
        finally:
            n = lib.axon_stop_nrt_profile(str(output_dir).encode())
            print(f"profile: {n} ntff file(s) written to {output_dir}")

    mod = types.ModuleType("antenv.axon_hooks")
    _holder = {"h": _hook}
    mod.set_axon_ntff_profile_hook = lambda h: _holder.__setitem__("h", h)
    mod.get_axon_ntff_profile_hook = lambda: _holder["h"]
    sys.modules["antenv.axon_hooks"] = mod

    # avoid the S3/Fish artifact upload in the trace post-processing path
    import concourse.bass_utils as bu

    bu.upload_artifacts = lambda tmpdir: str(tmpdir)


def _pick_nt0(counts):
    """Smallest NT0 (multiple of 128) such that the overflow of every
    expert beyond NT0 fits in the 8 per-core 128-token top-up slots.
    Compare against the no-top-up template (pad all to max count)."""
    cmax = int(counts.max())
    nt_plain = max(512, -(-cmax // 128) * 128)
    best = None
    for nt0 in range(512, nt_plain + 128, 128):
        need = sum(-(-max(0, int(c) - nt0) // NT1) for c in counts)
        if need <= N_CORES:
            best = nt0
            break
    if best is None or best + NT1 >= nt_plain + NT1:
        best = nt_plain  # top-ups unused (gate=0 padding)
    return best


def kernel(**inputs):
    from concourse.bass_utils import run_bass_kernel_spmd
    import ml_dtypes

    if TRACE:
        _install_ntff_hook()

    x = np.asarray(inputs["x"], np.float32)
    w1 = np.asarray(inputs["w1"], np.float32)
    b1 = np.asarray(inputs["b1"], np.float32)
    w2 = np.asarray(inputs["w2"], np.float32)
    b2 = np.asarray(inputs["b2"], np.float32)
    wg = np.asarray(inputs["wg"], np.float32)
    bg = np.asarray(inputs["bg"], np.float32)

    T = x.shape[0] * x.shape[1]
    xf = x.reshape(T, D)

    # ---- host gating (fp64): logits -> top-2 (jax.lax.top_k tie order:
    # lower index wins -> stable argsort on -logits) -> softmax over top-2.
    logits = xf.astype(np.float64) @ wg.astype(np.float64) + bg.astype(np.float64)
    order = np.argsort(-logits, axis=1, kind="stable")
    top_idx = order[:, :TOP_K]                      # [T, K]
    top_vals = np.take_along_axis(logits, top_idx, axis=1)
    gwts = np.exp(top_vals - top_vals.max(axis=1, keepdims=True))
    gwts = gwts / gwts.sum(axis=1, keepdims=True)   # [T, K]

    # ---- dispatch: sort slots (t, k) by expert; per-expert contiguous runs.
    flat_expert = top_idx.ravel()                   # slot s = t*K + k
    perm = np.argsort(flat_expert, kind="stable")   # slots grouped by expert
    counts = np.bincount(flat_expert, minlength=E)
    cum = np.concatenate([[0], np.cumsum(counts)])
    slot_tok = perm // TOP_K                        # token of each sorted slot
    gates_sorted = gwts.ravel()[perm].astype(np.float32)

    NT0 = _pick_nt0(counts)
    NT = NT0 + NT1
    NTG = NT // 128
    chunks = _chunk_plan(NT0)
    offs = [sum(chunks[:i]) for i in range(len(chunks) + 1)]

    io_dtype = ml_dtypes.bfloat16
    w1_io = w1.astype(io_dtype)
    w2_io = w2.astype(io_dtype)

    # top-up assignment: expert e's slots beyond NT0, chopped into
    # 128-blocks, each block -> one core's slot1. record: (core, e, lo, n)
    topups = []
    next_core = 0
    for e in range(E):
        n = int(counts[e])
        for lo in range(NT0, n, NT1):
            nb = min(NT1, n - lo)
            assert next_core < N_CORES, "top-up slots exhausted"
            topups.append((next_core, e, lo, nb))
            next_core += 1
    topup_by_core = {c: (e, lo, nb) for (c, e, lo, nb) in topups}

    def permute_x(xt):
        # xt [D, NT] -> [128, KD*NT]: per chunk, (kc, token) contiguous
        xr = xt.reshape(KD, 128, NT)
        parts = [
            xr[:, :, offs[c]:offs[c + 1]].transpose(1, 0, 2).reshape(128, -1)
            for c in range(len(chunks))
        ]
        return np.ascontiguousarray(np.concatenate(parts, axis=1))

    def pack_w1(e):
        # [128, FB*KD*128] fb-major: col = fb*KD*128 + kc*128 + j
        w = w1_io[e].reshape(KD, 128, FB, 128)       # [kc, p, fb, j]
        return np.ascontiguousarray(
            w.transpose(1, 2, 0, 3).reshape(128, FB * KD * 128))

    def pack_w2(e):
        return np.ascontiguousarray(
            w2_io[e].reshape(FB, 128, D).transpose(1, 0, 2).reshape(128, FB * D))

    def pack_b1(e):
        return np.ascontiguousarray(b1[e].reshape(FB, 128).T)

    in_maps = []
    for c in range(N_CORES):
        n0 = min(int(counts[c]), NT0)
        toks0 = slot_tok[cum[c]:cum[c] + n0]
        xt = np.zeros((D, NT), io_dtype)
        xt[:, :n0] = xf[toks0].astype(io_dtype).T
        gate = np.zeros(NT, np.float32)
        gate[:n0] = gates_sorted[cum[c]:cum[c] + n0]
        if c in topup_by_core:
            te, lo, nb = topup_by_core[c]
            tt = slot_tok[cum[te] + lo:cum[te] + lo + nb]
            xt[:, NT0:NT0 + nb] = xf[tt].astype(io_dtype).T
            gate[NT0:NT0 + nb] = gates_sorted[cum[te] + lo:cum[te] + lo + nb]
            eb = te
        else:
            eb = 0  # unused slot1: gate=0 rows, any weights
        in_maps.append({
            "xp": permute_x(xt),
            "w1a": pack_w1(c), "w2a": pack_w2(c), "b1a": pack_b1(c),
            "w1b": pack_w1(eb), "w2b": pack_w2(eb), "b1b": pack_b1(eb),
            "gate2": np.ascontiguousarray(gate.reshape(NTG, 128).T),
        })

    def run_device():
        if NT0 not in _PROGRAM_CACHE:
            _PROGRAM_CACHE[NT0] = _build_program(NT0)
        nc = _PROGRAM_CACHE[NT0]
        res = run_bass_kernel_spmd(nc, in_maps, list(range(N_CORES)), trace=TRACE)
        if TRACE and res.exec_time_ns is not None:
            print(f"HW exec time: {res.exec_time_ns} ns")
        return [res.results[c]["y"] for c in range(N_CORES)]

    try:
        try:
            y_cores = run_device()
        except Exception:
            # transient device errors (e.g. NRT exec-unit unrecoverable)
            # are usually gone on retry with a freshly built program
            _PROGRAM_CACHE.clear()
            y_cores = run_device()
    except Exception as exc:
        # last resort: identical math on the host so the result is still
        # correct even if the accelerator path is down
        import sys
        print(f"device path failed twice ({exc!r}); computing FFN on host",
              file=sys.stderr)
        out_slots = np.zeros((T * TOP_K, D), np.float32)
        for e in range(E):
            n = int(counts[e])
            toks = slot_tok[cum[e]:cum[e] + n]
            h = np.maximum(xf[toks] @ w1[e] + b1[e], 0.0)
            y = (h @ w2[e]) * gates_sorted[cum[e]:cum[e] + n, None]
            out_slots[perm[cum[e]:cum[e] + n]] = y.astype(np.float32)
        out = out_slots.reshape(T, TOP_K, D).sum(axis=1)
        combine = np.zeros((T, E), np.float32)
        np.put_along_axis(combine, top_idx, gwts.astype(np.float32), axis=1)
        out += combine @ b2
        return out.reshape(B, S, D).astype(np.float32)

    # ---- unshard: scatter slots back, sum the K slots per token, add b2.
    out_slots = np.zeros((T * TOP_K, D), np.float32)
    for c in range(N_CORES):
        n0 = min(int(counts[c]), NT0)
        out_slots[perm[cum[c]:cum[c] + n0]] = \
            y_cores[c][:n0].astype(np.float32)
    for (c, e, lo, nb) in topups:
        out_slots[perm[cum[e] + lo:cum[e] + lo + nb]] = \
            y_cores[c][NT0:NT0 + nb].astype(np.float32)
    out = out_slots.reshape(T, TOP_K, D).sum(axis=1)

    # combine @ b2 (gate-weighted expert output biases)
    combine = np.zeros((T, E), np.float32)
    np.put_along_axis(combine, top_idx, gwts.astype(np.float32), axis=1)
    out += combine @ b2

    return out.reshape(B, S, D).astype(np.float32)

